# revision 12
# baseline (speedup 1.0000x reference)
"""Trainium2 Bass kernel for a dense transformer block (B=2,S=2048,E=768,H=12,D=64,F=3072).

Sharding: 8 cores = 2 batch groups x 4 cores. Within a batch group each core
computes attention for 3 of the 12 heads over the full sequence (uniform causal
work across cores), partial output projections are combined with a 4-core
ReduceScatter, and each core then runs the FFN on its 512 rows with replicated
FFN weights. Matmuls run in float32r (TF32-like) with fp32 accumulation.
"""

import sys

if "/opt/trn_rl_repo" not in sys.path:
    sys.path.insert(0, "/opt/trn_rl_repo")

import numpy as np

import concourse.bacc as bacc
import concourse.mybir as mybir
import concourse.tile as tile
from concourse.bass_utils import run_bass_kernel_spmd

B, S, E, H, D, F = 2, 2048, 768, 12, 64, 3072
NCORES = 8
R = 4          # cores per batch group
HPC = 3        # heads per core
MYR = S // R   # rows per core after reduce-scatter (512)
EC = E // 128  # 6 e-chunks
SC = S // 128  # 16 s-chunks of 128
SB = S // 256  # 8 s-blocks of 256
FC = F // 128  # 24 f-chunks
VW = 256       # padded V width (3 heads x 65 = 195 -> 256)

f32 = mybir.dt.float32
f32r = mybir.dt.float32r
AF = mybir.ActivationFunctionType
ALU = mybir.AluOpType

MASK_BIAS = -50.0

_CACHE = {}


def _declare_io(nc):
    t = {}

    def inp(name, shape):
        t[name] = nc.dram_tensor(name, list(shape), f32, kind="ExternalInput").ap()

    inp("xb", (128, SC * E))          # x[b] rows-chunked
    inp("xmy", (128, 4 * E))          # my 512 rows of x[b]
    inp("wq", (128, EC * 192))        # Wq_eff for my heads, e-chunked
    inp("bq_row", (1, 192))
    inp("wv", (128, EC * VW))         # Wv_aug for my heads (65 cols/head, padded)
    inp("bv_row", (1, VW))
    inp("wo0", (128, E))              # Wo rows for my heads 0,1
    inp("wo1", (64, E))               # Wo rows for my head 2
    inp("bo_row", (1, E))             # bo on group-rank 0, else zeros
    inp("mb", (128, SC))              # per-key mask bias, t-chunked
    inp("d0", (128, SB * 256))        # bad-row diag injection, lower diag chunk
    inp("d1", (128, SB * 256))        # bad-row diag injection, upper diag chunk
    inp("m0", (128, 256))             # causal 0/1 mask, lower diag chunk
    inp("m1", (128, 256))             # causal 0/1 mask, upper diag chunk
    inp("ident", (128, 128))
    inp("ones_row", (1, 512))
    inp("w1", (128, EC * F))          # W1_eff e-chunked
    inp("b1c", (128, FC))
    inp("w2", (128, FC * F))          # W2 f1-chunked
    inp("b2c", (128, FC))
    inp("w3", (128, FC * E))          # W3 f-chunked
    inp("b3_row", (1, E))
    t["out"] = nc.dram_tensor("out", [128, 4 * E], f32, kind="ExternalOutput").ap()
    return t


def _layernorm_chunk(nc, pool, x_chunk, out_chunk):
    """LN a [128, 768] fp32 chunk into out_chunk (f32r), eps=1e-5."""
    stats = pool.tile([128, 12], f32, tag="ln_stats")
    nc.vector.bn_stats(stats[:, 0:6], x_chunk[:, 0:384])
    nc.vector.bn_stats(stats[:, 6:12], x_chunk[:, 384:768])
    mv = pool.tile([128, 2], f32, tag="ln_mv")
    nc.vector.bn_aggr(mv[:], stats[:])
    veps = pool.tile([128, 1], f32, tag="ln_veps")
    nc.vector.tensor_scalar_add(veps[:], mv[:, 1:2], 1e-5)
    sig = pool.tile([128, 1], f32, tag="ln_sig")
    nc.scalar.sqrt(sig[:], veps[:])
    rsig = pool.tile([128, 1], f32, tag="ln_rsig")
    nc.vector.reciprocal(rsig[:], sig[:])
    negmurs = pool.tile([128, 1], f32, tag="ln_negmurs")
    nc.vector.scalar_tensor_tensor(
        negmurs[:], mv[:, 0:1], -1.0, rsig[:], ALU.mult, ALU.mult
    )
    nc.scalar.activation(out_chunk, x_chunk, AF.Identity, bias=negmurs[:], scale=rsig[:])


def _build_body(tc, t):
    nc = tc.nc

    with tc.tile_pool(name="constp", bufs=1) as constp, \
         tc.tile_pool(name="lnstat", bufs=4) as lnstat, \
         tc.tile_pool(name="dramp", bufs=1, space="DRAM") as dramp:
        proj_bounce = dramp.tile([S, E], f32)
        rs_out = dramp.tile([MYR, E], f32)
        ident = constp.tile([128, 128], f32r)
        nc.sync.dma_start(ident[:], t["ident"][:].bitcast(f32r))
        ones_row = constp.tile([1, 512], f32r)
        nc.sync.dma_start(ones_row[:], t["ones_row"][:].bitcast(f32r))
        b1c = constp.tile([128, FC], f32)
        nc.sync.dma_start(b1c[:], t["b1c"][:])
        b2c = constp.tile([128, FC], f32)
        nc.sync.dma_start(b2c[:], t["b2c"][:])
        b3_row = constp.tile([1, E], f32r)
        nc.sync.dma_start(b3_row[:], t["b3_row"][:].bitcast(f32r))

        with tc.tile_pool(name="aconstp", bufs=1) as aconstp:
            m0 = aconstp.tile([128, 256], f32r)
            nc.sync.dma_start(m0[:], t["m0"][:].bitcast(f32r))
            m1 = aconstp.tile([128, 256], f32r)
            nc.sync.dma_start(m1[:], t["m1"][:].bitcast(f32r))
            mb = aconstp.tile([128, SC], f32)
            nc.sync.dma_start(mb[:], t["mb"][:])
            d0 = aconstp.tile([128, SB * 256], f32r)
            nc.sync.dma_start(d0[:], t["d0"][:].bitcast(f32r))
            d1 = aconstp.tile([128, SB * 256], f32r)
            nc.sync.dma_start(d1[:], t["d1"][:].bitcast(f32r))
            wq = aconstp.tile([128, EC * 192], f32r)
            nc.sync.dma_start(wq[:], t["wq"][:].bitcast(f32r))
            bq_row = aconstp.tile([1, 192], f32r)
            nc.sync.dma_start(bq_row[:], t["bq_row"][:].bitcast(f32r))
            wv = aconstp.tile([128, EC * VW], f32r)
            nc.sync.dma_start(wv[:], t["wv"][:].bitcast(f32r))
            bv_row = aconstp.tile([1, VW], f32r)
            nc.sync.dma_start(bv_row[:], t["bv_row"][:].bitcast(f32r))
            wo0 = aconstp.tile([128, E], f32r)
            nc.sync.dma_start(wo0[:], t["wo0"][:].bitcast(f32r))
            wo1 = aconstp.tile([64, E], f32r)
            nc.sync.dma_start(wo1[:], t["wo1"][:].bitcast(f32r))
            bo_row = aconstp.tile([1, E], f32r)
            nc.sync.dma_start(bo_row[:], t["bo_row"][:].bitcast(f32r))

            wq3 = wq[:].rearrange("p (e m) -> p e m", e=EC)
            wv3 = wv[:].rearrange("p (e m) -> p e m", e=EC)

            with tc.tile_pool(name="qvp", bufs=1) as qvp:
                # ---------------- Phase A: LN1(x_b) + transpose -> ln1T ----
                with tc.tile_pool(name="ln1Tp", bufs=1) as ln1Tp:
                    ln1T = ln1Tp.tile([128, EC * S], f32r)
                    ln1T3 = ln1T[:].rearrange("p (e s) -> p e s", e=EC)
                    with tc.tile_pool(name="xp", bufs=1) as xp, \
                         tc.tile_pool(name="lnxp", bufs=3) as lnxp, \
                         tc.tile_pool(name="psA", bufs=4, space="PSUM") as psA:
                        x_sb = xp.tile([128, SC * E], f32)
                        nc.sync.dma_start(x_sb[:], t["xb"][:])
                        x3 = x_sb[:].rearrange("p (s e) -> p s e", s=SC)
                        for sc in range(SC):
                            lnx = lnxp.tile([128, E], f32r, tag="lnx")
                            _layernorm_chunk(nc, lnstat, x3[:, sc, :], lnx[:])
                            for ec in range(EC):
                                pt = psA.tile([128, 128], f32r, tag="pt")
                                nc.tensor.transpose(
                                    pt[:], lnx[:, ec * 128:(ec + 1) * 128], ident[:]
                                )
                                nc.scalar.copy(
                                    ln1T3[:, ec, sc * 128:(sc + 1) * 128], pt[:]
                                )

                    # ------------- Phase B: Q^T and V projections ----------
                    q01T = qvp.tile([128, S], f32r)
                    q2T = qvp.tile([64, S], f32r)
                    v_sb = qvp.tile([128, SC * VW], f32r)
                    v3 = v_sb[:].rearrange("p (s v) -> p s v", s=SC)
                    with tc.tile_pool(name="psQ", bufs=3, space="PSUM") as psQ, \
                         tc.tile_pool(name="evacp", bufs=3) as evacp:
                        # Q^T: out [d(2 heads->128 | 1 head->64), s]
                        for g in range(2):
                            m = 128 if g == 0 else 64
                            for s4 in range(4):
                                pq = psQ.tile([128, 512], f32, tag="pq")
                                for ec in range(EC):
                                    nc.tensor.matmul(
                                        pq[:m, :],
                                        wq3[:, ec, g * 128:g * 128 + m],
                                        ln1T3[:, ec, s4 * 512:(s4 + 1) * 512],
                                        start=(ec == 0), stop=False,
                                    )
                                nc.tensor.matmul(
                                    pq[:m, :],
                                    bq_row[:, g * 128:g * 128 + m],
                                    ones_row[:],
                                    start=False, stop=True,
                                )
                                dst = q01T if g == 0 else q2T
                                nc.scalar.copy(
                                    dst[:m, s4 * 512:(s4 + 1) * 512], pq[:m, :]
                                )
                        # V: out [t, 65*3 padded to 256]
                        for sc in range(SC):
                            pv = psQ.tile([128, 512], f32, tag="pq")
                            for ec in range(EC):
                                nc.tensor.matmul(
                                    pv[:, :VW],
                                    ln1T3[:, ec, sc * 128:(sc + 1) * 128],
                                    wv3[:, ec, :],
                                    start=(ec == 0), stop=False,
                                )
                            nc.tensor.matmul(
                                pv[:, :VW],
                                ones_row[:, :128],
                                bv_row[:],
                                start=False, stop=True,
                            )
                            nc.scalar.copy(v3[:, sc, :], pv[:, :VW])

                # ---------------- Phase C: attention -----------------------
                oT01 = qvp.tile([128, S], f32r)
                oT2 = qvp.tile([64, S], f32r)
                d03 = d0[:].rearrange("p (j s) -> p j s", j=SB)
                d13 = d1[:].rearrange("p (j s) -> p j s", j=SB)
                with tc.tile_pool(name="pexp", bufs=4) as pexp, \
                     tc.tile_pool(name="osml", bufs=3) as osml, \
                     tc.tile_pool(name="psP", bufs=4, space="PSUM") as psP, \
                     tc.tile_pool(name="psO", bufs=2, space="PSUM") as psO, \
                     tc.tile_pool(name="psBc", bufs=2, space="PSUM") as psBc:
                    for hh in range(HPC):
                        qT = q01T[64 * hh:64 * (hh + 1), :] if hh < 2 else q2T[:, :]
                        oT = oT01 if hh < 2 else oT2
                        op_off = 64 * hh if hh < 2 else 0
                        for j in range(SB):
                            po = psO.tile([128, 256], f32, tag="po")
                            ntc = 2 * j + 2
                            for tc_i in range(ntc):
                                pp = psP.tile([128, 256], f32, tag="pp")
                                nc.tensor.matmul(
                                    pp[:],
                                    qT[:, tc_i * 128:(tc_i + 1) * 128],
                                    qT[:, j * 256:(j + 1) * 256],
                                    start=True, stop=True,
                                )
                                pe = pexp.tile([128, 256], f32r, tag="pe")
                                nc.scalar.activation(
                                    pe[:], pp[:], AF.Exp,
                                    bias=mb[:, tc_i:tc_i + 1], scale=0.125,
                                )
                                if tc_i == ntc - 2:
                                    nc.vector.tensor_mul(pe[:], pe[:], m0[:])
                                    nc.vector.tensor_add(pe[:], pe[:], d03[:, j, :])
                                elif tc_i == ntc - 1:
                                    nc.vector.tensor_mul(pe[:], pe[:], m1[:])
                                    nc.vector.tensor_add(pe[:], pe[:], d13[:, j, :])
                                nc.tensor.matmul(
                                    po[:65, :],
                                    v3[:, tc_i, 65 * hh:65 * hh + 65],
                                    pe[:],
                                    start=(tc_i == 0), stop=(tc_i == ntc - 1),
                                )
                            # softmax divide: oT[:, jblock] = po[:64]/po[64]
                            recip = osml.tile([1, 256], f32r, tag="recip")
                            with nc.allow_low_precision(reason="softmax recip f32r"):
                                nc.vector.reciprocal(recip[:], po[64:65, :])
                            pb = psBc.tile([64, 256], f32, tag="pb")
                            nc.tensor.matmul(
                                pb[:], ones_row[:, :64], recip[:],
                                start=True, stop=True,
                            )
                            sb_o = osml.tile([64, 256], f32r, tag="sb_o")
                            nc.scalar.copy(sb_o[:], po[:64, :])
                            sb_b = osml.tile([64, 256], f32r, tag="sb_b")
                            nc.scalar.copy(sb_b[:], pb[:])
                            nc.vector.tensor_mul(
                                oT[op_off:op_off + 64, j * 256:(j + 1) * 256],
                                sb_o[:], sb_b[:],
                            )

                # ---------------- Phase D: Wo projection + ReduceScatter ---
                with tc.tile_pool(name="psW", bufs=4, space="PSUM") as psW, \
                     tc.tile_pool(name="projp", bufs=4) as projp:
                    for st in range(SC):
                        for half in range(2):
                            pw = psW.tile([128, 384], f32, tag="pw")
                            nc.tensor.matmul(
                                pw[:],
                                oT01[:, st * 128:(st + 1) * 128],
                                wo0[:, half * 384:(half + 1) * 384],
                                start=True, stop=False,
                            )
                            nc.tensor.matmul(
                                pw[:],
                                oT2[:, st * 128:(st + 1) * 128],
                                wo1[:, half * 384:(half + 1) * 384],
                                start=False, stop=False,
                            )
                            nc.tensor.matmul(
                                pw[:],
                                ones_row[:, :128],
                                bo_row[:, half * 384:(half + 1) * 384],
                                start=False, stop=True,
                            )
                            prj = projp.tile([128, 384], f32, tag="prj")
                            nc.scalar.copy(prj[:], pw[:])
                            nc.sync.dma_start(
                                proj_bounce[st * 128:(st + 1) * 128,
                                            half * 384:(half + 1) * 384],
                                prj[:],
                            )

        nc.gpsimd.collective_compute(
            "ReduceScatter",
            ALU.add,
            replica_groups=[[0, 1, 2, 3], [4, 5, 6, 7]],
            ins=[proj_bounce[:, :].opt()],
            outs=[rs_out[:, :].opt()],
        )

        # ---------------- Phase E: residual + LN2 + FFN --------------------
        with tc.tile_pool(name="yp", bufs=1) as yp:
            y1 = yp.tile([128, 4 * E], f32)
            y13 = y1[:].rearrange("p (c e) -> p c e", c=4)
            ylnT = yp.tile([128, EC * MYR], f32r)
            ylnT3 = ylnT[:].rearrange("p (e s) -> p e s", e=EC)
            with tc.tile_pool(name="rsp", bufs=1) as rsp, \
                 tc.tile_pool(name="lnyp", bufs=2) as lnyp, \
                 tc.tile_pool(name="psE", bufs=4, space="PSUM") as psE:
                rs_sb = rsp.tile([128, 4 * E], f32)
                nc.sync.dma_start(
                    rs_sb[:].rearrange("p (c e) -> p c e", c=4),
                    rs_out[:, :].rearrange("(c p) e -> p c e", p=128),
                )
                xmy_sb = rsp.tile([128, 4 * E], f32)
                nc.sync.dma_start(xmy_sb[:], t["xmy"][:])
                nc.vector.tensor_add(y1[:], rs_sb[:], xmy_sb[:])
                for rc in range(4):
                    lny = lnyp.tile([128, E], f32r, tag="lny")
                    _layernorm_chunk(nc, lnstat, y13[:, rc, :], lny[:])
                    for ec in range(EC):
                        pt = psE.tile([128, 128], f32r, tag="pt2")
                        nc.tensor.transpose(
                            pt[:], lny[:, ec * 128:(ec + 1) * 128], ident[:]
                        )
                        nc.scalar.copy(
                            ylnT3[:, ec, rc * 128:(rc + 1) * 128], pt[:]
                        )

            w13 = t["w1"][:].rearrange("p (e f) -> p e f", e=EC)
            w23 = t["w2"][:].rearrange("p (f g) -> p f g", f=FC)
            w33 = t["w3"][:].rearrange("p (f e) -> p f e", f=FC)

            with tc.tile_pool(name="ffp", bufs=1) as ffp:
                h1T = ffp.tile([128, FC * MYR], f32r)
                h13 = h1T[:].rearrange("p (f s) -> p f s", f=FC)
                h2T = ffp.tile([128, FC * MYR], f32r)
                h23 = h2T[:].rearrange("p (f s) -> p f s", f=FC)

                with tc.tile_pool(name="w1p", bufs=3) as w1p, \
                     tc.tile_pool(name="w2p", bufs=4) as w2p, \
                     tc.tile_pool(name="psF1", bufs=2, space="PSUM") as psF1, \
                     tc.tile_pool(name="psF2", bufs=4, space="PSUM") as psF2:
                    # h1T = relu(W1^T @ yln^T + b1)
                    for fc in range(FC):
                        w1t = w1p.tile([128, EC * 128], f32r, tag="w1t")
                        nc.sync.dma_start(
                            w1t[:].rearrange("p (e f) -> p e f", e=EC),
                            w13[:, :, fc * 128:(fc + 1) * 128].bitcast(f32r),
                        )
                        w1t3 = w1t[:].rearrange("p (e f) -> p e f", e=EC)
                        pf = psF1.tile([128, 512], f32, tag="pf")
                        for ec in range(EC):
                            nc.tensor.matmul(
                                pf[:],
                                w1t3[:, ec, :],
                                ylnT3[:, ec, :],
                                start=(ec == 0), stop=(ec == EC - 1),
                            )
                        nc.scalar.activation(
                            h13[:, fc, :], pf[:], AF.Relu,
                            bias=b1c[:, fc:fc + 1], scale=1.0,
                        )
                    # h2T = relu(W2^T @ h1T + b2)
                    for f2b in range(6):
                        ph2 = [
                            psF2.tile([128, 512], f32, tag="ph2", name=f"ph2_{f2b}_{k}")
                            for k in range(4)
                        ]
                        for f1c in range(FC):
                            w2t = w2p.tile([128, 512], f32r, tag="w2t")
                            nc.sync.dma_start(
                                w2t[:],
                                w23[:, f1c, f2b * 512:(f2b + 1) * 512].bitcast(f32r),
                            )
                            for k in range(4):
                                nc.tensor.matmul(
                                    ph2[k][:],
                                    w2t[:, k * 128:(k + 1) * 128],
                                    h13[:, f1c, :],
                                    start=(f1c == 0), stop=(f1c == FC - 1),
                                )
                        for k in range(4):
                            fc2 = f2b * 4 + k
                            nc.scalar.activation(
                                h23[:, fc2, :], ph2[k][:], AF.Relu,
                                bias=b2c[:, fc2:fc2 + 1], scale=1.0,
                            )

                # out = h2 @ W3 + b3 + y1
                with tc.tile_pool(name="outp", bufs=1) as outp, \
                     tc.tile_pool(name="w3p", bufs=3) as w3p, \
                     tc.tile_pool(name="psF3", bufs=1, space="PSUM") as psF3:
                    out_sb = outp.tile([128, 4 * E], f32)
                    out3 = out_sb[:].rearrange("p (c e) -> p c e", c=4)
                    p3 = [
                        psF3.tile([128, 384], f32, tag=f"p3_{st}_{hf}",
                                  name=f"p3_{st}_{hf}")
                        for st in range(4) for hf in range(2)
                    ]
                    for fc in range(FC):
                        w3t = w3p.tile([128, E], f32r, tag="w3t")
                        nc.sync.dma_start(w3t[:], w33[:, fc, :].bitcast(f32r))
                        for st in range(4):
                            for hf in range(2):
                                nc.tensor.matmul(
                                    p3[st * 2 + hf][:],
                                    h23[:, fc, st * 128:(st + 1) * 128],
                                    w3t[:, hf * 384:(hf + 1) * 384],
                                    start=(fc == 0), stop=False,
                                )
                    for st in range(4):
                        for hf in range(2):
                            nc.tensor.matmul(
                                p3[st * 2 + hf][:],
                                ones_row[:, :128],
                                b3_row[:, hf * 384:(hf + 1) * 384],
                                start=False, stop=True,
                            )
                            nc.vector.tensor_add(
                                out3[:, st, hf * 384:(hf + 1) * 384],
                                p3[st * 2 + hf][:],
                                y13[:, st, hf * 384:(hf + 1) * 384],
                            )
                    nc.sync.dma_start(t["out"][:], out_sb[:])


def _build():
    if "nc" in _CACHE:
        return _CACHE["nc"]
    nc = bacc.Bacc("TRN2", target_bir_lowering=False, debug=False,
                   num_devices=NCORES)
    t = _declare_io(nc)
    with tile.TileContext(nc) as tc:
        _build_body(tc, t)
    nc.compile()
    _CACHE["nc"] = nc
    return nc


def _chunk_rows(a, p=128):
    """[N, M] -> [p, N//p, M] -> [p, (N//p)*M] row-chunk packing."""
    n, m = a.shape
    return np.ascontiguousarray(
        a.reshape(n // p, p, m).transpose(1, 0, 2).reshape(p, -1)
    )


def _prep_in_maps(inputs):
    x = np.asarray(inputs["x"], np.float32)
    Wq = np.asarray(inputs["Wq"], np.float32)
    bq = np.asarray(inputs["bq"], np.float32)
    Wv = np.asarray(inputs["Wv"], np.float32)
    bv = np.asarray(inputs["bv"], np.float32)
    Wo = np.asarray(inputs["Wo"], np.float32)
    bo = np.asarray(inputs["bo"], np.float32)
    ln1_g = np.asarray(inputs["ln1_g"], np.float32)
    ln1_b = np.asarray(inputs["ln1_b"], np.float32)
    W1 = np.asarray(inputs["W1"], np.float32)
    b1 = np.asarray(inputs["b1"], np.float32)
    W2 = np.asarray(inputs["W2"], np.float32)
    b2 = np.asarray(inputs["b2"], np.float32)
    W3 = np.asarray(inputs["W3"], np.float32)
    b3 = np.asarray(inputs["b3"], np.float32)
    ln2_g = np.asarray(inputs["ln2_g"], np.float32)
    ln2_b = np.asarray(inputs["ln2_b"], np.float32)
    mask = np.asarray(inputs["input_mask"])

    # Fold LN affine params into the following projections (exact algebra).
    Wq_eff = Wq * ln1_g[None, :, None]
    bq_eff = bq + np.einsum("e,hed->hd", ln1_b, Wq)
    Wv_eff = Wv * ln1_g[None, :, None]
    bv_eff = bv + np.einsum("e,hed->hd", ln1_b, Wv)
    W1_eff = W1 * ln2_g[:, None]
    b1_eff = b1 + ln2_b @ W1

    w1_p = _chunk_rows(W1_eff)                      # [128, 6*3072]
    b1c = np.ascontiguousarray(b1_eff.reshape(FC, 128).T)
    w2_p = _chunk_rows(W2)                          # [128, 24*3072]
    b2c = np.ascontiguousarray(b2.reshape(FC, 128).T)
    w3_p = _chunk_rows(W3)                          # [128, 24*768]
    b3_row = b3.reshape(1, E)

    ident = np.eye(128, dtype=np.float32)
    ones_row = np.ones((1, 512), np.float32)

    # causal 0/1 masks for the two diagonal 128x256 chunks of a 256 s-block
    tl = np.arange(128)[:, None]
    sl = np.arange(256)[None, :]
    m0 = (tl <= sl).astype(np.float32)
    m1 = ((tl + 128) <= sl).astype(np.float32)

    in_maps = []
    for c in range(NCORES):
        b, r = c // R, c % R
        hs = [HPC * r + i for i in range(HPC)]

        xb = _chunk_rows(x[b])                      # [128, 16*768]
        xmy = _chunk_rows(x[b, MYR * r: MYR * (r + 1)])

        Wq_my = np.concatenate([Wq_eff[h] for h in hs], axis=1)   # [E, 192]
        bq_my = np.concatenate([bq_eff[h] for h in hs])           # [192]
        wq_p = _chunk_rows(Wq_my)
        bq_row = bq_my.reshape(1, 192)

        Wv_aug = np.zeros((E, VW), np.float32)
        bv_row = np.zeros((1, VW), np.float32)
        for i, h in enumerate(hs):
            Wv_aug[:, 65 * i: 65 * i + 64] = Wv_eff[h]
            bv_row[0, 65 * i: 65 * i + 64] = bv_eff[h]
            bv_row[0, 65 * i + 64] = 1.0
        wv_p = _chunk_rows(Wv_aug)

        wo0 = np.ascontiguousarray(Wo[hs[0] * D: hs[0] * D + 128])
        wo1 = np.ascontiguousarray(Wo[hs[2] * D: hs[2] * D + 64])
        bo_row = (bo if r == 0 else np.zeros_like(bo)).reshape(1, E)

        mbias = np.where(mask[b] == 0, MASK_BIAS, 0.0).astype(np.float32)
        mb_p = np.ascontiguousarray(mbias.reshape(SC, 128).T)     # [128, 16]

        bad = (np.cumsum(mask[b]) == 0).astype(np.float32)        # [S]
        d0_p = np.zeros((128, SB, 256), np.float32)
        d1_p = np.zeros((128, SB, 256), np.float32)
        i128 = np.arange(128)
        for j in range(SB):
            d0_p[i128, j, i128] = bad[256 * j + i128]
            d1_p[i128, j, 128 + i128] = bad[256 * j + 128 + i128]

        in_maps.append({
            "xb": xb, "xmy": xmy,
            "wq": wq_p, "bq_row": bq_row,
            "wv": wv_p, "bv_row": bv_row,
            "wo0": wo0, "wo1": wo1, "bo_row": bo_row,
            "mb": mb_p,
            "d0": d0_p.reshape(128, -1), "d1": d1_p.reshape(128, -1),
            "m0": m0, "m1": m1,
            "ident": ident, "ones_row": ones_row,
            "w1": w1_p, "b1c": b1c,
            "w2": w2_p, "b2c": b2c,
            "w3": w3_p, "b3_row": b3_row,
        })
    return in_maps


def _gather(results):
    y = np.empty((B, S, E), np.float32)
    for c in range(NCORES):
        b, r = c // R, c % R
        o = results[c]["out"].reshape(128, 4, E).transpose(1, 0, 2).reshape(MYR, E)
        y[b, MYR * r: MYR * (r + 1)] = o
    return y


def run(inputs, **spmd_kwargs):
    nc = _build()
    in_maps = _prep_in_maps(inputs)
    res = run_bass_kernel_spmd(nc, in_maps, core_ids=list(range(NCORES)),
                               **spmd_kwargs)
    return _gather(res.results), res


def kernel(**inputs) -> np.ndarray:
    y, _ = run(inputs)
    return y


# revision 15
# speedup vs baseline: 1.1370x; 1.1370x over previous
"""Trainium2 Bass kernel for a dense transformer block (B=2,S=2048,E=768,H=12,D=64,F=3072).

Sharding: 8 cores = 2 batch groups x 4 cores. Within a batch group each core
computes attention for 3 of the 12 heads over the full sequence (uniform causal
work across cores), partial output projections are combined with a 4-core
ReduceScatter, and each core then runs the FFN on its 512 rows with replicated
FFN weights. Matmuls run in float32r (TF32-like) with fp32 accumulation.
"""

import sys

if "/opt/trn_rl_repo" not in sys.path:
    sys.path.insert(0, "/opt/trn_rl_repo")

import numpy as np

import concourse.bacc as bacc
import concourse.mybir as mybir
import concourse.tile as tile
from concourse.bass_utils import run_bass_kernel_spmd

B, S, E, H, D, F = 2, 2048, 768, 12, 64, 3072
NCORES = 8
R = 4          # cores per batch group
HPC = 3        # heads per core
MYR = S // R   # rows per core after reduce-scatter (512)
EC = E // 128  # 6 e-chunks
SC = S // 128  # 16 s-chunks of 128
SB = S // 256  # 8 s-blocks of 256
FC = F // 128  # 24 f-chunks
VW = 256       # padded V width (3 heads x 65 = 195 -> 256)

f32 = mybir.dt.float32
f16 = mybir.dt.float16
AF = mybir.ActivationFunctionType
ALU = mybir.AluOpType

MASK_BIAS = -50.0
EXP_SHIFT = -8.0  # uniform exp shift; cancels in softmax, keeps fp16 in range

_CACHE = {}


def _declare_io(nc):
    t = {}

    F16_INPUTS = {"wq", "bq_row", "wv", "bv_row", "wo0", "wo1", "bo_row",
                  "d0", "d1", "m0", "m1", "ident", "ones_row",
                  "w1", "w2", "w3", "b3_row"}

    def inp(name, shape):
        dt = f16 if name in F16_INPUTS else f32
        t[name] = nc.dram_tensor(name, list(shape), dt, kind="ExternalInput").ap()

    inp("xb", (128, SC * E))          # x[b] rows-chunked
    inp("xmy", (128, 4 * E))          # my 512 rows of x[b]
    inp("wq", (128, EC * 192))        # Wq_eff for my heads, e-chunked
    inp("bq_row", (1, 192))
    inp("wv", (128, EC * VW))         # Wv_aug for my heads (65 cols/head, padded)
    inp("bv_row", (1, VW))
    inp("wo0", (128, E))              # Wo rows for my heads 0,1
    inp("wo1", (64, E))               # Wo rows for my head 2
    inp("bo_row", (1, E))             # bo on group-rank 0, else zeros
    inp("mb", (128, SC))              # per-key mask bias, t-chunked
    inp("d0", (128, SB * 256))        # bad-row diag injection, lower diag chunk
    inp("d1", (128, SB * 256))        # bad-row diag injection, upper diag chunk
    inp("m0", (128, 256))             # causal 0/1 mask, lower diag chunk
    inp("m1", (128, 256))             # causal 0/1 mask, upper diag chunk
    inp("ident", (128, 128))
    inp("ones_row", (1, 512))
    inp("w1", (128, EC * F))          # W1_eff e-chunked
    inp("b1c", (128, FC))
    inp("w2", (128, FC * F))          # W2 f1-chunked
    inp("b2c", (128, FC))
    inp("w3", (128, FC * E))          # W3 f-chunked
    inp("b3_row", (1, E))
    t["out"] = nc.dram_tensor("out", [128, 4 * E], f32, kind="ExternalOutput").ap()
    return t


def _layernorm_chunk(nc, pool, x_chunk, out_chunk):
    """LN a [128, 768] fp32 chunk into out_chunk (f16), eps=1e-5."""
    stats = pool.tile([128, 12], f32, tag="ln_stats")
    nc.vector.bn_stats(stats[:, 0:6], x_chunk[:, 0:384])
    nc.vector.bn_stats(stats[:, 6:12], x_chunk[:, 384:768])
    mv = pool.tile([128, 2], f32, tag="ln_mv")
    nc.vector.bn_aggr(mv[:], stats[:])
    veps = pool.tile([128, 1], f32, tag="ln_veps")
    nc.vector.tensor_scalar_add(veps[:], mv[:, 1:2], 1e-5)
    sig = pool.tile([128, 1], f32, tag="ln_sig")
    nc.scalar.sqrt(sig[:], veps[:])
    rsig = pool.tile([128, 1], f32, tag="ln_rsig")
    nc.vector.reciprocal(rsig[:], sig[:])
    negmurs = pool.tile([128, 1], f32, tag="ln_negmurs")
    nc.vector.scalar_tensor_tensor(
        negmurs[:], mv[:, 0:1], -1.0, rsig[:], ALU.mult, ALU.mult
    )
    nc.scalar.activation(out_chunk, x_chunk, AF.Identity, bias=negmurs[:], scale=rsig[:])


def _build_body(tc, t):
    nc = tc.nc

    with tc.tile_pool(name="constp", bufs=1) as constp, \
         tc.tile_pool(name="lnstat", bufs=4) as lnstat, \
         tc.tile_pool(name="dramp", bufs=1, space="DRAM") as dramp:
        proj_bounce = dramp.tile([S, E], f32)
        rs_out = dramp.tile([MYR, E], f32)
        ident = constp.tile([128, 128], f16)
        nc.sync.dma_start(ident[:], t["ident"][:])
        ones_row = constp.tile([1, 512], f16)
        nc.sync.dma_start(ones_row[:], t["ones_row"][:])
        b1c = constp.tile([128, FC], f32)
        nc.sync.dma_start(b1c[:], t["b1c"][:])
        b2c = constp.tile([128, FC], f32)
        nc.sync.dma_start(b2c[:], t["b2c"][:])
        b3_row = constp.tile([1, E], f16)
        nc.sync.dma_start(b3_row[:], t["b3_row"][:])

        with tc.tile_pool(name="aconstp", bufs=1) as aconstp:
            m0 = aconstp.tile([128, 256], f16)
            nc.sync.dma_start(m0[:], t["m0"][:])
            m1 = aconstp.tile([128, 256], f16)
            nc.sync.dma_start(m1[:], t["m1"][:])
            mb = aconstp.tile([128, SC], f32)
            nc.sync.dma_start(mb[:], t["mb"][:])
            d0 = aconstp.tile([128, SB * 256], f16)
            nc.sync.dma_start(d0[:], t["d0"][:])
            d1 = aconstp.tile([128, SB * 256], f16)
            nc.sync.dma_start(d1[:], t["d1"][:])
            wq = aconstp.tile([128, EC * 192], f16)
            nc.sync.dma_start(wq[:], t["wq"][:])
            bq_row = aconstp.tile([1, 192], f16)
            nc.sync.dma_start(bq_row[:], t["bq_row"][:])
            wv = aconstp.tile([128, EC * VW], f16)
            nc.sync.dma_start(wv[:], t["wv"][:])
            bv_row = aconstp.tile([1, VW], f16)
            nc.sync.dma_start(bv_row[:], t["bv_row"][:])
            wo0 = aconstp.tile([128, E], f16)
            nc.sync.dma_start(wo0[:], t["wo0"][:])
            wo1 = aconstp.tile([64, E], f16)
            nc.sync.dma_start(wo1[:], t["wo1"][:])
            bo_row = aconstp.tile([1, E], f16)
            nc.sync.dma_start(bo_row[:], t["bo_row"][:])

            wq3 = wq[:].rearrange("p (e m) -> p e m", e=EC)
            wv3 = wv[:].rearrange("p (e m) -> p e m", e=EC)

            with tc.tile_pool(name="qvp", bufs=1) as qvp:
                # ---------------- Phase A: LN1(x_b) + transpose -> ln1T ----
                with tc.tile_pool(name="ln1Tp", bufs=1) as ln1Tp:
                    ln1T = ln1Tp.tile([128, EC * S], f16)
                    ln1T3 = ln1T[:].rearrange("p (e s) -> p e s", e=EC)
                    with tc.tile_pool(name="xp", bufs=1) as xp, \
                         tc.tile_pool(name="lnxp", bufs=3) as lnxp, \
                         tc.tile_pool(name="psA", bufs=4, space="PSUM") as psA:
                        x_sb = xp.tile([128, SC * E], f32)
                        nc.sync.dma_start(x_sb[:], t["xb"][:])
                        x3 = x_sb[:].rearrange("p (s e) -> p s e", s=SC)
                        for sc in range(SC):
                            lnx = lnxp.tile([128, E], f16, tag="lnx")
                            _layernorm_chunk(nc, lnstat, x3[:, sc, :], lnx[:])
                            for ec in range(EC):
                                pt = psA.tile([128, 128], f16, tag="pt")
                                nc.tensor.transpose(
                                    pt[:], lnx[:, ec * 128:(ec + 1) * 128], ident[:]
                                )
                                nc.scalar.copy(
                                    ln1T3[:, ec, sc * 128:(sc + 1) * 128], pt[:]
                                )

                    # ------------- Phase B: Q^T and V projections ----------
                    q01T = qvp.tile([128, S], f16)
                    q2T = qvp.tile([64, S], f16)
                    v_sb = qvp.tile([128, SC * VW], f16)
                    v3 = v_sb[:].rearrange("p (s v) -> p s v", s=SC)
                    with tc.tile_pool(name="psQ", bufs=3, space="PSUM") as psQ, \
                         tc.tile_pool(name="evacp", bufs=3) as evacp:
                        # Q^T: out [d(2 heads->128 | 1 head->64), s]
                        for g in range(2):
                            m = 128 if g == 0 else 64
                            for s4 in range(4):
                                pq = psQ.tile([128, 512], f32, tag="pq")
                                for ec in range(EC):
                                    nc.tensor.matmul(
                                        pq[:m, :],
                                        wq3[:, ec, g * 128:g * 128 + m],
                                        ln1T3[:, ec, s4 * 512:(s4 + 1) * 512],
                                        start=(ec == 0), stop=False,
                                    )
                                nc.tensor.matmul(
                                    pq[:m, :],
                                    bq_row[:, g * 128:g * 128 + m],
                                    ones_row[:],
                                    start=False, stop=True,
                                )
                                dst = q01T if g == 0 else q2T
                                nc.scalar.copy(
                                    dst[:m, s4 * 512:(s4 + 1) * 512], pq[:m, :]
                                )
                        # V: out [t, 65*3 padded to 256]
                        for sc in range(SC):
                            pv = psQ.tile([128, 512], f32, tag="pq")
                            for ec in range(EC):
                                nc.tensor.matmul(
                                    pv[:, :VW],
                                    ln1T3[:, ec, sc * 128:(sc + 1) * 128],
                                    wv3[:, ec, :],
                                    start=(ec == 0), stop=False,
                                )
                            nc.tensor.matmul(
                                pv[:, :VW],
                                ones_row[:, :128],
                                bv_row[:],
                                start=False, stop=True,
                            )
                            nc.scalar.copy(v3[:, sc, :], pv[:, :VW])

                # ---------------- Phase C: attention -----------------------
                oT01 = qvp.tile([128, S], f16)
                oT2 = qvp.tile([64, S], f16)
                d03 = d0[:].rearrange("p (j s) -> p j s", j=SB)
                d13 = d1[:].rearrange("p (j s) -> p j s", j=SB)
                with tc.tile_pool(name="pexp", bufs=4) as pexp, \
                     tc.tile_pool(name="osml", bufs=3) as osml, \
                     tc.tile_pool(name="psP", bufs=4, space="PSUM") as psP, \
                     tc.tile_pool(name="psO", bufs=2, space="PSUM") as psO, \
                     tc.tile_pool(name="psBc", bufs=2, space="PSUM") as psBc:
                    for hh in range(HPC):
                        qT = q01T[64 * hh:64 * (hh + 1), :] if hh < 2 else q2T[:, :]
                        oT = oT01 if hh < 2 else oT2
                        op_off = 64 * hh if hh < 2 else 0
                        for j in range(SB):
                            po = psO.tile([128, 256], f32, tag="po")
                            ntc = 2 * j + 2
                            for tc_i in range(ntc):
                                pp = psP.tile([128, 256], f32, tag="pp")
                                nc.tensor.matmul(
                                    pp[:],
                                    qT[:, tc_i * 128:(tc_i + 1) * 128],
                                    qT[:, j * 256:(j + 1) * 256],
                                    start=True, stop=True,
                                )
                                pe = pexp.tile([128, 256], f16, tag="pe")
                                nc.scalar.activation(
                                    pe[:], pp[:], AF.Exp,
                                    bias=mb[:, tc_i:tc_i + 1], scale=0.125,
                                )
                                if tc_i == ntc - 2:
                                    nc.vector.tensor_mul(pe[:], pe[:], m0[:])
                                    nc.vector.tensor_add(pe[:], pe[:], d03[:, j, :])
                                elif tc_i == ntc - 1:
                                    nc.vector.tensor_mul(pe[:], pe[:], m1[:])
                                    nc.vector.tensor_add(pe[:], pe[:], d13[:, j, :])
                                nc.tensor.matmul(
                                    po[:65, :],
                                    v3[:, tc_i, 65 * hh:65 * hh + 65],
                                    pe[:],
                                    start=(tc_i == 0), stop=(tc_i == ntc - 1),
                                )
                            # softmax divide: oT[:, jblock] = po[:64]/po[64]
                            recip = osml.tile([1, 256], f16, tag="recip")
                            with nc.allow_low_precision(reason="softmax recip f16"):
                                nc.vector.reciprocal(recip[:], po[64:65, :])
                            pb = psBc.tile([64, 256], f32, tag="pb")
                            nc.tensor.matmul(
                                pb[:], ones_row[:, :64], recip[:],
                                start=True, stop=True,
                            )
                            sb_o = osml.tile([64, 256], f16, tag="sb_o")
                            nc.scalar.copy(sb_o[:], po[:64, :])
                            sb_b = osml.tile([64, 256], f16, tag="sb_b")
                            nc.scalar.copy(sb_b[:], pb[:])
                            nc.vector.tensor_mul(
                                oT[op_off:op_off + 64, j * 256:(j + 1) * 256],
                                sb_o[:], sb_b[:],
                            )

                # ---------------- Phase D: Wo projection + ReduceScatter ---
                with tc.tile_pool(name="psW", bufs=4, space="PSUM") as psW, \
                     tc.tile_pool(name="projp", bufs=4) as projp:
                    for st in range(SC):
                        for half in range(2):
                            pw = psW.tile([128, 384], f32, tag="pw")
                            nc.tensor.matmul(
                                pw[:],
                                oT01[:, st * 128:(st + 1) * 128],
                                wo0[:, half * 384:(half + 1) * 384],
                                start=True, stop=False,
                            )
                            nc.tensor.matmul(
                                pw[:],
                                oT2[:, st * 128:(st + 1) * 128],
                                wo1[:, half * 384:(half + 1) * 384],
                                start=False, stop=False,
                            )
                            nc.tensor.matmul(
                                pw[:],
                                ones_row[:, :128],
                                bo_row[:, half * 384:(half + 1) * 384],
                                start=False, stop=True,
                            )
                            prj = projp.tile([128, 384], f32, tag="prj")
                            nc.scalar.copy(prj[:], pw[:])
                            nc.sync.dma_start(
                                proj_bounce[st * 128:(st + 1) * 128,
                                            half * 384:(half + 1) * 384],
                                prj[:],
                            )

        nc.gpsimd.collective_compute(
            "ReduceScatter",
            ALU.add,
            replica_groups=[[0, 1, 2, 3], [4, 5, 6, 7]],
            ins=[proj_bounce[:, :].opt()],
            outs=[rs_out[:, :].opt()],
        )

        # ---------------- Phase E: residual + LN2 + FFN --------------------
        with tc.tile_pool(name="yp", bufs=1) as yp:
            y1 = yp.tile([128, 4 * E], f32)
            y13 = y1[:].rearrange("p (c e) -> p c e", c=4)
            ylnT = yp.tile([128, EC * MYR], f16)
            ylnT3 = ylnT[:].rearrange("p (e s) -> p e s", e=EC)
            with tc.tile_pool(name="rsp", bufs=1) as rsp, \
                 tc.tile_pool(name="lnyp", bufs=2) as lnyp, \
                 tc.tile_pool(name="psE", bufs=4, space="PSUM") as psE:
                rs_sb = rsp.tile([128, 4 * E], f32)
                # gpsimd queue: don't head-of-line-block FFN weight prefetch
                # on the sync queue behind the collective.
                nc.gpsimd.dma_start(
                    rs_sb[:].rearrange("p (c e) -> p c e", c=4),
                    rs_out[:, :].rearrange("(c p) e -> p c e", p=128),
                )
                xmy_sb = rsp.tile([128, 4 * E], f32)
                nc.sync.dma_start(xmy_sb[:], t["xmy"][:])
                nc.vector.tensor_add(y1[:], rs_sb[:], xmy_sb[:])
                for rc in range(4):
                    lny = lnyp.tile([128, E], f16, tag="lny")
                    _layernorm_chunk(nc, lnstat, y13[:, rc, :], lny[:])
                    for ec in range(EC):
                        pt = psE.tile([128, 128], f16, tag="pt2")
                        nc.tensor.transpose(
                            pt[:], lny[:, ec * 128:(ec + 1) * 128], ident[:]
                        )
                        nc.scalar.copy(
                            ylnT3[:, ec, rc * 128:(rc + 1) * 128], pt[:]
                        )

            w13 = t["w1"][:].rearrange("p (e f) -> p e f", e=EC)
            w23 = t["w2"][:].rearrange("p (f g) -> p f g", f=FC)
            w33 = t["w3"][:].rearrange("p (f e) -> p f e", f=FC)

            with tc.tile_pool(name="ffp", bufs=1) as ffp:
                h1T = ffp.tile([128, FC * MYR], f16)
                h13 = h1T[:].rearrange("p (f s) -> p f s", f=FC)
                h2T = ffp.tile([128, FC * MYR], f16)
                h23 = h2T[:].rearrange("p (f s) -> p f s", f=FC)

                with tc.tile_pool(name="w1p", bufs=3) as w1p, \
                     tc.tile_pool(name="w2p", bufs=4) as w2p, \
                     tc.tile_pool(name="psF1", bufs=2, space="PSUM") as psF1, \
                     tc.tile_pool(name="psF2", bufs=4, space="PSUM") as psF2:
                    # h1T = relu(W1^T @ yln^T + b1)
                    for fc in range(FC):
                        w1t = w1p.tile([128, EC * 128], f16, tag="w1t")
                        nc.sync.dma_start(
                            w1t[:].rearrange("p (e f) -> p e f", e=EC),
                            w13[:, :, fc * 128:(fc + 1) * 128],
                        )
                        w1t3 = w1t[:].rearrange("p (e f) -> p e f", e=EC)
                        pf = psF1.tile([128, 512], f32, tag="pf")
                        for ec in range(EC):
                            nc.tensor.matmul(
                                pf[:],
                                w1t3[:, ec, :],
                                ylnT3[:, ec, :],
                                start=(ec == 0), stop=(ec == EC - 1),
                            )
                        nc.scalar.activation(
                            h13[:, fc, :], pf[:], AF.Relu,
                            bias=b1c[:, fc:fc + 1], scale=1.0,
                        )
                    # h2T = relu(W2^T @ h1T + b2)
                    for f2b in range(6):
                        ph2 = [
                            psF2.tile([128, 512], f32, tag="ph2", name=f"ph2_{f2b}_{k}")
                            for k in range(4)
                        ]
                        for f1c in range(FC):
                            w2t = w2p.tile([128, 512], f16, tag="w2t")
                            nc.sync.dma_start(
                                w2t[:],
                                w23[:, f1c, f2b * 512:(f2b + 1) * 512],
                            )
                            for k in range(4):
                                nc.tensor.matmul(
                                    ph2[k][:],
                                    w2t[:, k * 128:(k + 1) * 128],
                                    h13[:, f1c, :],
                                    start=(f1c == 0), stop=(f1c == FC - 1),
                                )
                        for k in range(4):
                            fc2 = f2b * 4 + k
                            nc.scalar.activation(
                                h23[:, fc2, :], ph2[k][:], AF.Relu,
                                bias=b2c[:, fc2:fc2 + 1], scale=1.0,
                            )

                # out = h2 @ W3 + b3 + y1
                with tc.tile_pool(name="outp", bufs=1) as outp, \
                     tc.tile_pool(name="w3p", bufs=3) as w3p, \
                     tc.tile_pool(name="psF3", bufs=1, space="PSUM") as psF3:
                    out_sb = outp.tile([128, 4 * E], f32)
                    out3 = out_sb[:].rearrange("p (c e) -> p c e", c=4)
                    p3 = [
                        psF3.tile([128, 384], f32, tag=f"p3_{st}_{hf}",
                                  name=f"p3_{st}_{hf}")
                        for st in range(4) for hf in range(2)
                    ]
                    for fc in range(FC):
                        w3t = w3p.tile([128, E], f16, tag="w3t")
                        nc.sync.dma_start(w3t[:], w33[:, fc, :])
                        for st in range(4):
                            for hf in range(2):
                                nc.tensor.matmul(
                                    p3[st * 2 + hf][:],
                                    h23[:, fc, st * 128:(st + 1) * 128],
                                    w3t[:, hf * 384:(hf + 1) * 384],
                                    start=(fc == 0), stop=False,
                                )
                    for st in range(4):
                        for hf in range(2):
                            nc.tensor.matmul(
                                p3[st * 2 + hf][:],
                                ones_row[:, :128],
                                b3_row[:, hf * 384:(hf + 1) * 384],
                                start=False, stop=True,
                            )
                            nc.vector.tensor_add(
                                out3[:, st, hf * 384:(hf + 1) * 384],
                                p3[st * 2 + hf][:],
                                y13[:, st, hf * 384:(hf + 1) * 384],
                            )
                    nc.sync.dma_start(t["out"][:], out_sb[:])


def _build():
    if "nc" in _CACHE:
        return _CACHE["nc"]
    nc = bacc.Bacc("TRN2", target_bir_lowering=False, debug=False,
                   num_devices=NCORES)
    t = _declare_io(nc)
    with tile.TileContext(nc) as tc:
        _build_body(tc, t)
    nc.compile()
    _CACHE["nc"] = nc
    return nc


def _chunk_rows(a, p=128):
    """[N, M] -> [p, N//p, M] -> [p, (N//p)*M] row-chunk packing."""
    n, m = a.shape
    return np.ascontiguousarray(
        a.reshape(n // p, p, m).transpose(1, 0, 2).reshape(p, -1)
    )


def _prep_in_maps(inputs):
    x = np.asarray(inputs["x"], np.float32)
    Wq = np.asarray(inputs["Wq"], np.float32)
    bq = np.asarray(inputs["bq"], np.float32)
    Wv = np.asarray(inputs["Wv"], np.float32)
    bv = np.asarray(inputs["bv"], np.float32)
    Wo = np.asarray(inputs["Wo"], np.float32)
    bo = np.asarray(inputs["bo"], np.float32)
    ln1_g = np.asarray(inputs["ln1_g"], np.float32)
    ln1_b = np.asarray(inputs["ln1_b"], np.float32)
    W1 = np.asarray(inputs["W1"], np.float32)
    b1 = np.asarray(inputs["b1"], np.float32)
    W2 = np.asarray(inputs["W2"], np.float32)
    b2 = np.asarray(inputs["b2"], np.float32)
    W3 = np.asarray(inputs["W3"], np.float32)
    b3 = np.asarray(inputs["b3"], np.float32)
    ln2_g = np.asarray(inputs["ln2_g"], np.float32)
    ln2_b = np.asarray(inputs["ln2_b"], np.float32)
    mask = np.asarray(inputs["input_mask"])

    # Fold LN affine params into the following projections (exact algebra).
    Wq_eff = Wq * ln1_g[None, :, None]
    bq_eff = bq + np.einsum("e,hed->hd", ln1_b, Wq)
    Wv_eff = Wv * ln1_g[None, :, None]
    bv_eff = bv + np.einsum("e,hed->hd", ln1_b, Wv)
    W1_eff = W1 * ln2_g[:, None]
    b1_eff = b1 + ln2_b @ W1

    w1_p = _chunk_rows(W1_eff)                      # [128, 6*3072]
    b1c = np.ascontiguousarray(b1_eff.reshape(FC, 128).T)
    w2_p = _chunk_rows(W2)                          # [128, 24*3072]
    b2c = np.ascontiguousarray(b2.reshape(FC, 128).T)
    w3_p = _chunk_rows(W3)                          # [128, 24*768]
    b3_row = b3.reshape(1, E)

    ident = np.eye(128, dtype=np.float32)
    ones_row = np.ones((1, 512), np.float32)

    # causal 0/1 masks for the two diagonal 128x256 chunks of a 256 s-block
    tl = np.arange(128)[:, None]
    sl = np.arange(256)[None, :]
    m0 = (tl <= sl).astype(np.float32)
    m1 = ((tl + 128) <= sl).astype(np.float32)

    in_maps = []
    for c in range(NCORES):
        b, r = c // R, c % R
        hs = [HPC * r + i for i in range(HPC)]

        xb = _chunk_rows(x[b])                      # [128, 16*768]
        xmy = _chunk_rows(x[b, MYR * r: MYR * (r + 1)])

        Wq_my = np.concatenate([Wq_eff[h] for h in hs], axis=1)   # [E, 192]
        bq_my = np.concatenate([bq_eff[h] for h in hs])           # [192]
        wq_p = _chunk_rows(Wq_my)
        bq_row = bq_my.reshape(1, 192)

        Wv_aug = np.zeros((E, VW), np.float32)
        bv_row = np.zeros((1, VW), np.float32)
        for i, h in enumerate(hs):
            Wv_aug[:, 65 * i: 65 * i + 64] = Wv_eff[h]
            bv_row[0, 65 * i: 65 * i + 64] = bv_eff[h]
            bv_row[0, 65 * i + 64] = 1.0
        wv_p = _chunk_rows(Wv_aug)

        wo0 = np.ascontiguousarray(Wo[hs[0] * D: hs[0] * D + 128])
        wo1 = np.ascontiguousarray(Wo[hs[2] * D: hs[2] * D + 64])
        bo_row = (bo if r == 0 else np.zeros_like(bo)).reshape(1, E)

        mbias = np.where(mask[b] == 0, MASK_BIAS + EXP_SHIFT, EXP_SHIFT).astype(np.float32)
        mb_p = np.ascontiguousarray(mbias.reshape(SC, 128).T)     # [128, 16]

        bad = (np.cumsum(mask[b]) == 0).astype(np.float32) * np.float32(np.exp(EXP_SHIFT))
        d0_p = np.zeros((128, SB, 256), np.float32)
        d1_p = np.zeros((128, SB, 256), np.float32)
        i128 = np.arange(128)
        for j in range(SB):
            d0_p[i128, j, i128] = bad[256 * j + i128]
            d1_p[i128, j, 128 + i128] = bad[256 * j + 128 + i128]

        f16c = np.float16
        in_maps.append({
            "xb": xb, "xmy": xmy,
            "wq": wq_p.astype(f16c), "bq_row": bq_row.astype(f16c),
            "wv": wv_p.astype(f16c), "bv_row": bv_row.astype(f16c),
            "wo0": wo0.astype(f16c), "wo1": wo1.astype(f16c),
            "bo_row": bo_row.astype(f16c),
            "mb": mb_p,
            "d0": d0_p.reshape(128, -1).astype(f16c),
            "d1": d1_p.reshape(128, -1).astype(f16c),
            "m0": m0.astype(f16c), "m1": m1.astype(f16c),
            "ident": ident.astype(f16c), "ones_row": ones_row.astype(f16c),
            "w1": w1_p.astype(f16c), "b1c": b1c,
            "w2": w2_p.astype(f16c), "b2c": b2c,
            "w3": w3_p.astype(f16c), "b3_row": b3_row.astype(f16c),
        })
    return in_maps


def _gather(results):
    y = np.empty((B, S, E), np.float32)
    for c in range(NCORES):
        b, r = c // R, c % R
        o = results[c]["out"].reshape(128, 4, E).transpose(1, 0, 2).reshape(MYR, E)
        y[b, MYR * r: MYR * (r + 1)] = o
    return y


def run(inputs, **spmd_kwargs):
    nc = _build()
    in_maps = _prep_in_maps(inputs)
    res = run_bass_kernel_spmd(nc, in_maps, core_ids=list(range(NCORES)),
                               **spmd_kwargs)
    return _gather(res.results), res


def kernel(**inputs) -> np.ndarray:
    y, _ = run(inputs)
    return y
